# revision 27
# baseline (speedup 1.0000x reference)
"""Trainium2 Bass kernel for nn_LogOddsPerformanceTransformer.

Computes, for each element x of Xs:
    s   = log(x) - log(1-x)              (log-odds)
    idx = clip(searchsorted(bins, max(s, bins[0]), 'right') - 1, 0, NB-1)
    out = bins[idx]

bins is a uniform grid (linspace), so binning reduces to an affine floor
via the magic-number rounding trick.  The post-log chain is one fused
6-stage custom-DVE op producing the integer bin offset j, plus one
2-ALU tensor_scalar (on gpsimd) for the final affine:

    OP1:  j   = ((clip((a-b)*inv, -31, 32) + M) - M)   # M = 2^23+31
    TS:   out = (j - 0.5) * step

Clamp bounds -31/32 (instead of the exact bin edges -31.5/32.5) keep
sg + 31 >= 0 so the magic add always lands on the integer rounding grid
at 2^23; any clamp value inside the first/last bin gives the identical
bin index.

Data parallel over 8 NeuronCores; per core the 524288-element slice is
viewed as [128 x 4096].  Input DMAs (SP/HWDGE) use a ramped column-tile
grid so the activation engine is never starved; compute runs on an
independent column-chunk grid (ACT 2x Ln -> DVE fused op -> Pool TS);
output DMAs issue per compute chunk on the SP sequencer, which is idle
after the input DMAs and whose in-order semaphore waits match the chunk
completion order.
"""

import sys

sys.path.insert(0, "/opt/trn_rl_repo")

from contextlib import ExitStack

import numpy as np

import concourse.bass as bass
import concourse.tile as tile
from concourse import bacc, mybir
from concourse.bass_utils import run_bass_kernel_spmd

N = 4_194_304
NCORES = 8
NPER = N // NCORES  # 524288
P = 128
W = NPER // P  # 4096 columns per core

# --- tunables -------------------------------------------------------------
IN_TILES = (128, 256, 512, 896, 1152, 1152)  # ramped; sum = 4096
# compute grid; sum = 4096.  Chunk ends should align under tile prefix sums
# so a chunk never waits on a tile it doesn't cover.
CHUNKS = (128, 256, 512, 896, 640, 640, 512, 384, 128)
# groups of chunk indices sharing one a-pass / b-pass (Ln) activation op:
# merging late-kernel ops (whose data has long arrived) saves the ~185ns
# per-instruction activation init without hurting the pipeline ramp.
A_GROUPS = tuple((i,) for i in range(len(CHUNKS)))
B_GROUPS = tuple((i,) for i in range(len(CHUNKS)))
# out-DMA grid; boundaries must be a subset of the chunk prefix sums.
OUT_TILES = CHUNKS
# per-out issue engine names; None -> all "sync".  The second-to-last out on
# "scalar" (ACT sequencer, idle by then) overlaps the last one's SP issue.
OUT_ENGINES = ("sync",) * 7 + ("scalar", "sync")
TAIL_TS_ON_DVE = 99  # last k chunks run the final tensor_scalar on DVE not Pool
TAIL_HIPRI = 0  # last k chunks emit OP1+TS under tc.high_priority()
# --------------------------------------------------------------------------

f32 = mybir.dt.float32
Alu = mybir.AluOpType
Act = mybir.ActivationFunctionType

_BUILD_CACHE: dict[tuple, object] = {}


# --- custom DVE op --------------------------------------------------------
def _j_ref(in0, in1, s0, s1, imm2):
    f = np.float32
    d = (in0.astype(f) - in1.astype(f)).astype(f)
    sg = (d * f(s0)).astype(f)
    mx = np.maximum(sg, f(imm2)).astype(f)
    lat = f(f(1.0) - f(imm2))
    mn = np.minimum(mx, lat).astype(f)
    t1 = (mn + f(s1)).astype(f)
    return (t1 - f(s1)).astype(f)


def _register_ops():
    import concourse.dve_ops as dve_ops
    from concourse.dve_spec import (
        Spec,
        Src0,
        Src1,
        C0,
        C1,
        C2,
        One,
        maxx,
        minn,
        lower,
        _has_src1,
    )
    from concourse.dve_uop import DveOpSpec

    def reg(name, spec):
        if name in dve_ops._SUB_OPCODE_FOR_NAME:
            return next(op for op in dve_ops.OPS if op.name == name)
        row = max(dve_ops._SUB_OPCODE_FOR_NAME.values()) + 1
        assert row < 0x20
        dve_ops._SUB_OPCODE_FOR_NAME[name] = row
        shas = {}
        for ver in ("v3", "v4"):
            uops = lower(spec, ver=ver)
            shas[ver] = DveOpSpec(
                name=name, opcode=row, uops=uops, rd1_en=_has_src1(spec)
            ).sha(ver)
        op = dve_ops.DveOp(name, spec, subdim=False, uops_sha=shas)
        dve_ops.OPS.append(op)
        dve_ops.CUSTOM_DVE_SPECS[name] = spec
        return op

    # j = ((clip((a-b)*C0, C2, 1-C2) + C1) - C1);  C2 = -31 so 1-C2 = 32
    sg = (Src0 - Src1) * C0
    mn = minn(maxx(sg, C2), One - C2)
    body = (mn + C1) - C1
    return reg("LOGODDS_J_ANT", Spec(body=body, reference=_j_ref))


_OP1 = _register_ops()


def _constants(bins: np.ndarray):
    """Host-side constants; returns None if the fused path can't be used
    (non-uniform bins or grid where the magic offsets aren't exact)."""
    b64 = bins.astype(np.float64)
    nb = len(bins)
    if nb != 64:
        return None
    step = np.float32((b64[-1] - b64[0]) / (nb - 1))
    inv = np.float32((nb - 1) / (b64[-1] - b64[0]))
    # sigma = s*inv ; bin edges at sigma = b0*inv + k.  Require b0*inv = -31.5
    # (true for the symmetric linspace(-6,6,64) grid) so the fixed clamp
    # bounds/magic below are exact.
    if not np.isclose(float(b64[0]) * float(inv), -31.5, atol=1e-6):
        return None
    uniform = np.allclose(
        np.diff(b64), (b64[-1] - b64[0]) / (nb - 1), rtol=0, atol=1e-5
    )
    if not uniform:
        return None
    return (float(inv), float(step))


MAGIC = float(np.float32(2.0**23 + 31.0))
SIG_LO = -31.0  # imm2 of OP1; upper clamp is 1-imm2 = 32
HALF = 0.5


def _build(inv, step):
    assert sum(IN_TILES) == W and sum(CHUNKS) == W and sum(OUT_TILES) == W
    ccum = np.cumsum(CHUNKS)
    assert set(np.cumsum(OUT_TILES)) <= set(ccum), "OUT_TILES must nest in CHUNKS"

    return _build_body(inv, step)


def _retag_const_memsets(nc):
    """Move the framework preamble's const-AP memsets off the Pool engine:
    their GPSIMD Q7 launch overhead (95ns each, serialized) otherwise delays
    the kernel start barrier by ~0.5us.  The all-engine barrier drains every
    engine, so the memsets complete before any consumer regardless of
    engine."""
    for bb in nc.m.functions[0].blocks:
        for ins in bb.instructions:
            if (
                type(ins).__name__ == "InstMemset"
                and ins.engine == mybir.EngineType.Pool
            ):
                ins.engine = mybir.EngineType.DVE


def _build_body(inv, step):
    nc = bacc.Bacc("TRN2", target_bir_lowering=False, debug=False)
    xs = nc.dram_tensor("xs", [P, W], f32, kind="ExternalInput").ap()
    outs = nc.dram_tensor("out", [P, W], f32, kind="ExternalOutput").ap()

    with tile.TileContext(nc) as tc, ExitStack() as ctx:
        tmp = ctx.enter_context(tc.tile_pool(name="tmp", bufs=1))

        x = tmp.tile([P, W], f32, tag="x")
        a = tmp.tile([P, W], f32, tag="a")
        b = tmp.tile([P, W], f32, tag="b")
        j = tmp.tile([P, W], f32, tag="j")
        o = tmp.tile([P, W], f32, tag="o")

        # all input DMAs issued first (high priority) so the out DMAs never
        # starve later input tiles
        with tc.high_priority():
            off = 0
            for w in IN_TILES:
                sl = (slice(None), slice(off, off + w))
                nc.sync.dma_start(x[sl], xs[sl])
                off += w

        # scalar constants built with DVE memsets (idle engine) so no
        # const-pool Memset gates the start barrier
        bias0 = tmp.tile([P, 1], f32, tag="bias0")
        bias1 = tmp.tile([P, 1], f32, tag="bias1")
        half_ap = tmp.tile([P, 1], f32, tag="half")
        step_ap = tmp.tile([P, 1], f32, tag="step")
        nc.vector.memset(bias0[:], 0.0)
        nc.vector.memset(bias1[:], 1.0)
        nc.vector.memset(half_ap[:], HALF)
        nc.vector.memset(step_ap[:], step)
        # warmup: forces the Ln act-table load to run during the DMA ramp
        # instead of gating the first real activation
        warm = tmp.tile([P, 1], f32, tag="warm")
        nc.scalar.activation(warm[:], bias1[:], Act.Ln, bias0[:])

        NCH = len(CHUNKS)
        ccum = [0] + list(np.cumsum(CHUNKS))
        assert sorted(i for g in A_GROUPS for i in g) == list(range(NCH))
        assert sorted(i for g in B_GROUPS for i in g) == list(range(NCH))
        a_first = {g[0]: g for g in A_GROUPS}
        b_first = {g[0]: g for g in B_GROUPS}
        out_cum = list(np.cumsum(OUT_TILES))
        for ci in range(NCH):
            if ci in a_first:
                g = a_first[ci]
                gsl = (slice(None), slice(ccum[g[0]], ccum[g[-1] + 1]))
                nc.scalar.activation(a[gsl], x[gsl], Act.Ln, bias0[:])
            if ci in b_first:
                g = b_first[ci]
                gsl = (slice(None), slice(ccum[g[0]], ccum[g[-1] + 1]))
                nc.scalar.activation(b[gsl], x[gsl], Act.Ln, bias1[:], -1.0)
            off, off2 = ccum[ci], ccum[ci + 1]
            sl = (slice(None), slice(off, off2))
            nc.vector._custom_dve(
                _OP1, out=j[sl], in0=a[sl], in1=b[sl], s0=inv, s1=MAGIC, imm2=SIG_LO
            )
            ts_eng = nc.vector if ci >= NCH - TAIL_TS_ON_DVE else nc.gpsimd
            ts_eng.tensor_scalar(
                o[sl], j[sl], half_ap[:], step_ap[:], Alu.subtract, Alu.mult
            )
            if off2 in out_cum:
                oi = out_cum.index(off2)
                prev = 0 if oi == 0 else out_cum[oi - 1]
                osl = (slice(None), slice(prev, off2))
                eng = "sync" if OUT_ENGINES is None else OUT_ENGINES[oi]
                getattr(nc, eng).dma_start(outs[osl], o[osl])

    _retag_const_memsets(nc)
    nc.compile()
    return nc


def build(bins: np.ndarray):
    key = _constants(bins)
    if key is None:
        raise NotImplementedError("unsupported bins for this kernel")
    if key not in _BUILD_CACHE:
        _BUILD_CACHE[key] = _build(*key)
    return _BUILD_CACHE[key]


def make_in_maps(Xs: np.ndarray):
    shards = Xs.reshape(NCORES, P, W)
    return [{"xs": shards[c]} for c in range(NCORES)]


def kernel(Xs: np.ndarray, bins: np.ndarray) -> np.ndarray:
    Xs = np.asarray(Xs, dtype=np.float32)
    bins = np.asarray(bins, dtype=np.float32)
    nc = build(bins)
    res = run_bass_kernel_spmd(nc, make_in_maps(Xs), core_ids=list(range(NCORES)))
    out = np.concatenate([r["out"].reshape(-1) for r in res.results])
    return out.astype(np.float32)


# revision 29
# speedup vs baseline: 1.0180x; 1.0180x over previous
"""Trainium2 Bass kernel for nn_LogOddsPerformanceTransformer.

Computes, for each element x of Xs:
    s   = log(x) - log(1-x)              (log-odds)
    idx = clip(searchsorted(bins, max(s, bins[0]), 'right') - 1, 0, NB-1)
    out = bins[idx]

bins is a uniform grid (linspace), so binning reduces to an affine floor
via the magic-number rounding trick.  The post-log chain is one fused
6-stage custom-DVE op producing the integer bin offset j, plus one
2-ALU tensor_scalar (on gpsimd) for the final affine:

    OP1:  j   = ((clip((a-b)*inv, -31, 32) + M) - M)   # M = 2^23+31
    TS:   out = (j - 0.5) * step

Clamp bounds -31/32 (instead of the exact bin edges -31.5/32.5) keep
sg + 31 >= 0 so the magic add always lands on the integer rounding grid
at 2^23; any clamp value inside the first/last bin gives the identical
bin index.

Data parallel over 8 NeuronCores; per core the 524288-element slice is
viewed as [128 x 4096].  Input DMAs (SP/HWDGE) use a ramped column-tile
grid so the activation engine is never starved; compute runs on an
independent column-chunk grid (ACT 2x Ln -> DVE fused op -> Pool TS);
output DMAs issue per compute chunk on the SP sequencer, which is idle
after the input DMAs and whose in-order semaphore waits match the chunk
completion order.
"""

import sys

sys.path.insert(0, "/opt/trn_rl_repo")

from contextlib import ExitStack

import numpy as np

import concourse.bass as bass
import concourse.tile as tile
from concourse import bacc, mybir
from concourse.bass_utils import run_bass_kernel_spmd

N = 4_194_304
NCORES = 8
NPER = N // NCORES  # 524288
P = 128
W = NPER // P  # 4096 columns per core

# --- tunables -------------------------------------------------------------
IN_TILES = (128, 256, 512, 896, 1152, 1152)  # ramped; sum = 4096
# compute grid; sum = 4096.  Chunk ends should align under tile prefix sums
# so a chunk never waits on a tile it doesn't cover.
CHUNKS = (128, 256, 512, 896, 640, 640, 512, 320, 192)
# groups of chunk indices sharing one a-pass / b-pass (Ln) activation op:
# merging late-kernel ops (whose data has long arrived) saves the ~185ns
# per-instruction activation init without hurting the pipeline ramp.
A_GROUPS = tuple((i,) for i in range(len(CHUNKS)))
B_GROUPS = tuple((i,) for i in range(len(CHUNKS)))
# out-DMA grid; boundaries must be a subset of the chunk prefix sums.
OUT_TILES = CHUNKS
# per-out issue engine names; None -> all "sync".  The second-to-last out on
# "scalar" (ACT sequencer, idle by then) overlaps the last one's SP issue.
OUT_ENGINES = ("sync",) * 7 + ("scalar", "sync")
TAIL_TS_ON_DVE = 99  # last k chunks run the final tensor_scalar on DVE not Pool
TAIL_HIPRI = 0  # last k chunks emit OP1+TS under tc.high_priority()
# --------------------------------------------------------------------------

f32 = mybir.dt.float32
Alu = mybir.AluOpType
Act = mybir.ActivationFunctionType

_BUILD_CACHE: dict[tuple, object] = {}


# --- custom DVE op --------------------------------------------------------
def _j_ref(in0, in1, s0, s1, imm2):
    f = np.float32
    d = (in0.astype(f) - in1.astype(f)).astype(f)
    sg = (d * f(s0)).astype(f)
    mx = np.maximum(sg, f(imm2)).astype(f)
    lat = f(f(1.0) - f(imm2))
    mn = np.minimum(mx, lat).astype(f)
    t1 = (mn + f(s1)).astype(f)
    return (t1 - f(s1)).astype(f)


def _register_ops():
    import concourse.dve_ops as dve_ops
    from concourse.dve_spec import (
        Spec,
        Src0,
        Src1,
        C0,
        C1,
        C2,
        One,
        maxx,
        minn,
        lower,
        _has_src1,
    )
    from concourse.dve_uop import DveOpSpec

    def reg(name, spec):
        if name in dve_ops._SUB_OPCODE_FOR_NAME:
            return next(op for op in dve_ops.OPS if op.name == name)
        row = max(dve_ops._SUB_OPCODE_FOR_NAME.values()) + 1
        assert row < 0x20
        dve_ops._SUB_OPCODE_FOR_NAME[name] = row
        shas = {}
        for ver in ("v3", "v4"):
            uops = lower(spec, ver=ver)
            shas[ver] = DveOpSpec(
                name=name, opcode=row, uops=uops, rd1_en=_has_src1(spec)
            ).sha(ver)
        op = dve_ops.DveOp(name, spec, subdim=False, uops_sha=shas)
        dve_ops.OPS.append(op)
        dve_ops.CUSTOM_DVE_SPECS[name] = spec
        return op

    # j = ((clip((a-b)*C0, C2, 1-C2) + C1) - C1);  C2 = -31 so 1-C2 = 32
    sg = (Src0 - Src1) * C0
    mn = minn(maxx(sg, C2), One - C2)
    body = (mn + C1) - C1
    return reg("LOGODDS_J_ANT", Spec(body=body, reference=_j_ref))


_OP1 = _register_ops()


def _constants(bins: np.ndarray):
    """Host-side constants; returns None if the fused path can't be used
    (non-uniform bins or grid where the magic offsets aren't exact)."""
    b64 = bins.astype(np.float64)
    nb = len(bins)
    if nb != 64:
        return None
    step = np.float32((b64[-1] - b64[0]) / (nb - 1))
    inv = np.float32((nb - 1) / (b64[-1] - b64[0]))
    # sigma = s*inv ; bin edges at sigma = b0*inv + k.  Require b0*inv = -31.5
    # (true for the symmetric linspace(-6,6,64) grid) so the fixed clamp
    # bounds/magic below are exact.
    if not np.isclose(float(b64[0]) * float(inv), -31.5, atol=1e-6):
        return None
    uniform = np.allclose(
        np.diff(b64), (b64[-1] - b64[0]) / (nb - 1), rtol=0, atol=1e-5
    )
    if not uniform:
        return None
    return (float(inv), float(step))


MAGIC = float(np.float32(2.0**23 + 31.0))
SIG_LO = -31.0  # imm2 of OP1; upper clamp is 1-imm2 = 32
HALF = 0.5


def _build(inv, step):
    assert sum(IN_TILES) == W and sum(CHUNKS) == W and sum(OUT_TILES) == W
    ccum = np.cumsum(CHUNKS)
    assert set(np.cumsum(OUT_TILES)) <= set(ccum), "OUT_TILES must nest in CHUNKS"

    return _build_body(inv, step)


def _retag_const_memsets(nc):
    """Strip the framework preamble's const-AP memsets when nothing in the
    kernel references those const tensors (this kernel passes all activation
    biases and tensor_scalar operands as its own APs or immediates).  The
    memsets otherwise gate the kernel start barrier by ~0.3-0.5us.  If any
    instruction does reference a const AP, fall back to retagging the memsets
    from Pool (95ns Q7 launch each) to the cheaper DVE engine."""
    fn = nc.m.functions[0]
    referenced = any(
        "memref='const" in str(arg)
        for bb in fn.blocks
        for ins in bb.instructions
        for arg in (getattr(ins, "ins", None) or [])
    )
    for bb in fn.blocks:
        dead = [
            ins
            for ins in list(bb.instructions)
            if type(ins).__name__ == "InstMemset"
            and "memref='const" in str(ins.outs[0])
        ]
        for ins in dead:
            if referenced:
                if ins.engine == mybir.EngineType.Pool:
                    ins.engine = mybir.EngineType.DVE
            else:
                bb.instructions.remove(ins)


def _build_body(inv, step):
    nc = bacc.Bacc("TRN2", target_bir_lowering=False, debug=False)
    xs = nc.dram_tensor("xs", [P, W], f32, kind="ExternalInput").ap()
    outs = nc.dram_tensor("out", [P, W], f32, kind="ExternalOutput").ap()

    with tile.TileContext(nc) as tc, ExitStack() as ctx:
        tmp = ctx.enter_context(tc.tile_pool(name="tmp", bufs=1))

        x = tmp.tile([P, W], f32, tag="x")
        a = tmp.tile([P, W], f32, tag="a")
        b = tmp.tile([P, W], f32, tag="b")
        j = tmp.tile([P, W], f32, tag="j")
        o = tmp.tile([P, W], f32, tag="o")

        # all input DMAs issued first (high priority) so the out DMAs never
        # starve later input tiles
        with tc.high_priority():
            off = 0
            for w in IN_TILES:
                sl = (slice(None), slice(off, off + w))
                nc.sync.dma_start(x[sl], xs[sl])
                off += w

        # scalar constants built with DVE memsets (idle engine) so no
        # const-pool Memset gates the start barrier
        bias0 = tmp.tile([P, 1], f32, tag="bias0")
        bias1 = tmp.tile([P, 1], f32, tag="bias1")
        half_ap = tmp.tile([P, 1], f32, tag="half")
        step_ap = tmp.tile([P, 1], f32, tag="step")
        nc.vector.memset(bias0[:], 0.0)
        nc.vector.memset(bias1[:], 1.0)
        nc.vector.memset(half_ap[:], HALF)
        nc.vector.memset(step_ap[:], step)
        # warmup: forces the Ln act-table load to run during the DMA ramp
        # instead of gating the first real activation
        warm = tmp.tile([P, 1], f32, tag="warm")
        nc.scalar.activation(warm[:], bias1[:], Act.Ln, bias0[:])

        NCH = len(CHUNKS)
        ccum = [0] + list(np.cumsum(CHUNKS))
        assert sorted(i for g in A_GROUPS for i in g) == list(range(NCH))
        assert sorted(i for g in B_GROUPS for i in g) == list(range(NCH))
        a_first = {g[0]: g for g in A_GROUPS}
        b_first = {g[0]: g for g in B_GROUPS}
        out_cum = list(np.cumsum(OUT_TILES))
        for ci in range(NCH):
            if ci in a_first:
                g = a_first[ci]
                gsl = (slice(None), slice(ccum[g[0]], ccum[g[-1] + 1]))
                nc.scalar.activation(a[gsl], x[gsl], Act.Ln, bias0[:])
            if ci in b_first:
                g = b_first[ci]
                gsl = (slice(None), slice(ccum[g[0]], ccum[g[-1] + 1]))
                nc.scalar.activation(b[gsl], x[gsl], Act.Ln, bias1[:], -1.0)
            off, off2 = ccum[ci], ccum[ci + 1]
            sl = (slice(None), slice(off, off2))
            nc.vector._custom_dve(
                _OP1, out=j[sl], in0=a[sl], in1=b[sl], s0=inv, s1=MAGIC, imm2=SIG_LO
            )
            ts_eng = nc.vector if ci >= NCH - TAIL_TS_ON_DVE else nc.gpsimd
            ts_eng.tensor_scalar(
                o[sl], j[sl], half_ap[:], step_ap[:], Alu.subtract, Alu.mult
            )
            if off2 in out_cum:
                oi = out_cum.index(off2)
                prev = 0 if oi == 0 else out_cum[oi - 1]
                osl = (slice(None), slice(prev, off2))
                eng = "sync" if OUT_ENGINES is None else OUT_ENGINES[oi]
                getattr(nc, eng).dma_start(outs[osl], o[osl])

    _retag_const_memsets(nc)
    nc.compile()
    return nc


def build(bins: np.ndarray):
    key = _constants(bins)
    if key is None:
        raise NotImplementedError("unsupported bins for this kernel")
    if key not in _BUILD_CACHE:
        _BUILD_CACHE[key] = _build(*key)
    return _BUILD_CACHE[key]


def make_in_maps(Xs: np.ndarray):
    shards = Xs.reshape(NCORES, P, W)
    return [{"xs": shards[c]} for c in range(NCORES)]


def kernel(Xs: np.ndarray, bins: np.ndarray) -> np.ndarray:
    Xs = np.asarray(Xs, dtype=np.float32)
    bins = np.asarray(bins, dtype=np.float32)
    nc = build(bins)
    res = run_bass_kernel_spmd(nc, make_in_maps(Xs), core_ids=list(range(NCORES)))
    out = np.concatenate([r["out"].reshape(-1) for r in res.results])
    return out.astype(np.float32)


# revision 30
# speedup vs baseline: 1.0215x; 1.0034x over previous
"""Trainium2 Bass kernel for nn_LogOddsPerformanceTransformer.

Computes, for each element x of Xs:
    s   = log(x) - log(1-x)              (log-odds)
    idx = clip(searchsorted(bins, max(s, bins[0]), 'right') - 1, 0, NB-1)
    out = bins[idx]

bins is a uniform grid (linspace), so binning reduces to an affine floor
via the magic-number rounding trick.  The post-log chain is one fused
6-stage custom-DVE op producing the integer bin offset j, plus one
2-ALU tensor_scalar (on gpsimd) for the final affine:

    OP1:  j   = ((clip((a-b)*inv, -31, 32) + M) - M)   # M = 2^23+31
    TS:   out = (j - 0.5) * step

Clamp bounds -31/32 (instead of the exact bin edges -31.5/32.5) keep
sg + 31 >= 0 so the magic add always lands on the integer rounding grid
at 2^23; any clamp value inside the first/last bin gives the identical
bin index.

Data parallel over 8 NeuronCores; per core the 524288-element slice is
viewed as [128 x 4096].  Input DMAs (SP/HWDGE) use a ramped column-tile
grid so the activation engine is never starved; compute runs on an
independent column-chunk grid (ACT 2x Ln -> DVE fused op -> Pool TS);
output DMAs issue per compute chunk on the SP sequencer, which is idle
after the input DMAs and whose in-order semaphore waits match the chunk
completion order.
"""

import sys

sys.path.insert(0, "/opt/trn_rl_repo")

from contextlib import ExitStack

import numpy as np

import concourse.bass as bass
import concourse.tile as tile
from concourse import bacc, mybir
from concourse.bass_utils import run_bass_kernel_spmd

N = 4_194_304
NCORES = 8
NPER = N // NCORES  # 524288
P = 128
W = NPER // P  # 4096 columns per core

# --- tunables -------------------------------------------------------------
IN_TILES = (128, 256, 512, 896, 1152, 1152)  # ramped; sum = 4096
# compute grid; sum = 4096.  Chunk ends should align under tile prefix sums
# so a chunk never waits on a tile it doesn't cover.
CHUNKS = (128, 256, 512, 896, 672, 608, 480, 288, 256)
# groups of chunk indices sharing one a-pass / b-pass (Ln) activation op:
# merging late-kernel ops (whose data has long arrived) saves the ~185ns
# per-instruction activation init without hurting the pipeline ramp.
A_GROUPS = tuple((i,) for i in range(len(CHUNKS)))
B_GROUPS = tuple((i,) for i in range(len(CHUNKS)))
# out-DMA grid; boundaries must be a subset of the chunk prefix sums.
OUT_TILES = CHUNKS
# per-out issue engine names; None -> all "sync".  The second-to-last out on
# "scalar" (ACT sequencer, idle by then) overlaps the last one's SP issue.
OUT_ENGINES = ("sync",) * 7 + ("scalar", "sync")
TAIL_TS_ON_DVE = 99  # last k chunks run the final tensor_scalar on DVE not Pool
TAIL_HIPRI = 0  # last k chunks emit OP1+TS under tc.high_priority()
# --------------------------------------------------------------------------

f32 = mybir.dt.float32
Alu = mybir.AluOpType
Act = mybir.ActivationFunctionType

_BUILD_CACHE: dict[tuple, object] = {}


# --- custom DVE op --------------------------------------------------------
def _j_ref(in0, in1, s0, s1, imm2):
    f = np.float32
    d = (in0.astype(f) - in1.astype(f)).astype(f)
    sg = (d * f(s0)).astype(f)
    mx = np.maximum(sg, f(imm2)).astype(f)
    lat = f(f(1.0) - f(imm2))
    mn = np.minimum(mx, lat).astype(f)
    t1 = (mn + f(s1)).astype(f)
    return (t1 - f(s1)).astype(f)


def _register_ops():
    import concourse.dve_ops as dve_ops
    from concourse.dve_spec import (
        Spec,
        Src0,
        Src1,
        C0,
        C1,
        C2,
        One,
        maxx,
        minn,
        lower,
        _has_src1,
    )
    from concourse.dve_uop import DveOpSpec

    def reg(name, spec):
        if name in dve_ops._SUB_OPCODE_FOR_NAME:
            return next(op for op in dve_ops.OPS if op.name == name)
        row = max(dve_ops._SUB_OPCODE_FOR_NAME.values()) + 1
        assert row < 0x20
        dve_ops._SUB_OPCODE_FOR_NAME[name] = row
        shas = {}
        for ver in ("v3", "v4"):
            uops = lower(spec, ver=ver)
            shas[ver] = DveOpSpec(
                name=name, opcode=row, uops=uops, rd1_en=_has_src1(spec)
            ).sha(ver)
        op = dve_ops.DveOp(name, spec, subdim=False, uops_sha=shas)
        dve_ops.OPS.append(op)
        dve_ops.CUSTOM_DVE_SPECS[name] = spec
        return op

    # j = ((clip((a-b)*C0, C2, 1-C2) + C1) - C1);  C2 = -31 so 1-C2 = 32
    sg = (Src0 - Src1) * C0
    mn = minn(maxx(sg, C2), One - C2)
    body = (mn + C1) - C1
    return reg("LOGODDS_J_ANT", Spec(body=body, reference=_j_ref))


_OP1 = _register_ops()


def _constants(bins: np.ndarray):
    """Host-side constants; returns None if the fused path can't be used
    (non-uniform bins or grid where the magic offsets aren't exact)."""
    b64 = bins.astype(np.float64)
    nb = len(bins)
    if nb != 64:
        return None
    step = np.float32((b64[-1] - b64[0]) / (nb - 1))
    inv = np.float32((nb - 1) / (b64[-1] - b64[0]))
    # sigma = s*inv ; bin edges at sigma = b0*inv + k.  Require b0*inv = -31.5
    # (true for the symmetric linspace(-6,6,64) grid) so the fixed clamp
    # bounds/magic below are exact.
    if not np.isclose(float(b64[0]) * float(inv), -31.5, atol=1e-6):
        return None
    uniform = np.allclose(
        np.diff(b64), (b64[-1] - b64[0]) / (nb - 1), rtol=0, atol=1e-5
    )
    if not uniform:
        return None
    return (float(inv), float(step))


MAGIC = float(np.float32(2.0**23 + 31.0))
SIG_LO = -31.0  # imm2 of OP1; upper clamp is 1-imm2 = 32
HALF = 0.5


def _build(inv, step):
    assert sum(IN_TILES) == W and sum(CHUNKS) == W and sum(OUT_TILES) == W
    ccum = np.cumsum(CHUNKS)
    assert set(np.cumsum(OUT_TILES)) <= set(ccum), "OUT_TILES must nest in CHUNKS"

    return _build_body(inv, step)


def _retag_const_memsets(nc):
    """Strip the framework preamble's const-AP memsets when nothing in the
    kernel references those const tensors (this kernel passes all activation
    biases and tensor_scalar operands as its own APs or immediates).  The
    memsets otherwise gate the kernel start barrier by ~0.3-0.5us.  If any
    instruction does reference a const AP, fall back to retagging the memsets
    from Pool (95ns Q7 launch each) to the cheaper DVE engine."""
    fn = nc.m.functions[0]
    referenced = any(
        "memref='const" in str(arg)
        for bb in fn.blocks
        for ins in bb.instructions
        for arg in (getattr(ins, "ins", None) or [])
    )
    for bb in fn.blocks:
        dead = [
            ins
            for ins in list(bb.instructions)
            if type(ins).__name__ == "InstMemset"
            and "memref='const" in str(ins.outs[0])
        ]
        for ins in dead:
            if referenced:
                if ins.engine == mybir.EngineType.Pool:
                    ins.engine = mybir.EngineType.DVE
            else:
                bb.instructions.remove(ins)


def _build_body(inv, step):
    nc = bacc.Bacc("TRN2", target_bir_lowering=False, debug=False)
    xs = nc.dram_tensor("xs", [P, W], f32, kind="ExternalInput").ap()
    outs = nc.dram_tensor("out", [P, W], f32, kind="ExternalOutput").ap()

    with tile.TileContext(nc) as tc, ExitStack() as ctx:
        tmp = ctx.enter_context(tc.tile_pool(name="tmp", bufs=1))

        x = tmp.tile([P, W], f32, tag="x")
        a = tmp.tile([P, W], f32, tag="a")
        b = tmp.tile([P, W], f32, tag="b")
        j = tmp.tile([P, W], f32, tag="j")
        o = tmp.tile([P, W], f32, tag="o")

        # all input DMAs issued first (high priority) so the out DMAs never
        # starve later input tiles
        with tc.high_priority():
            off = 0
            for w in IN_TILES:
                sl = (slice(None), slice(off, off + w))
                nc.sync.dma_start(x[sl], xs[sl])
                off += w

        # scalar constants built with DVE memsets (idle engine) so no
        # const-pool Memset gates the start barrier
        bias0 = tmp.tile([P, 1], f32, tag="bias0")
        bias1 = tmp.tile([P, 1], f32, tag="bias1")
        half_ap = tmp.tile([P, 1], f32, tag="half")
        step_ap = tmp.tile([P, 1], f32, tag="step")
        nc.vector.memset(bias0[:], 0.0)
        nc.vector.memset(bias1[:], 1.0)
        nc.vector.memset(half_ap[:], HALF)
        nc.vector.memset(step_ap[:], step)
        # warmup: forces the Ln act-table load to run during the DMA ramp
        # instead of gating the first real activation
        warm = tmp.tile([P, 1], f32, tag="warm")
        nc.scalar.activation(warm[:], bias1[:], Act.Ln, bias0[:])

        NCH = len(CHUNKS)
        ccum = [0] + list(np.cumsum(CHUNKS))
        assert sorted(i for g in A_GROUPS for i in g) == list(range(NCH))
        assert sorted(i for g in B_GROUPS for i in g) == list(range(NCH))
        a_first = {g[0]: g for g in A_GROUPS}
        b_first = {g[0]: g for g in B_GROUPS}
        out_cum = list(np.cumsum(OUT_TILES))
        for ci in range(NCH):
            if ci in a_first:
                g = a_first[ci]
                gsl = (slice(None), slice(ccum[g[0]], ccum[g[-1] + 1]))
                nc.scalar.activation(a[gsl], x[gsl], Act.Ln, bias0[:])
            if ci in b_first:
                g = b_first[ci]
                gsl = (slice(None), slice(ccum[g[0]], ccum[g[-1] + 1]))
                nc.scalar.activation(b[gsl], x[gsl], Act.Ln, bias1[:], -1.0)
            off, off2 = ccum[ci], ccum[ci + 1]
            sl = (slice(None), slice(off, off2))
            nc.vector._custom_dve(
                _OP1, out=j[sl], in0=a[sl], in1=b[sl], s0=inv, s1=MAGIC, imm2=SIG_LO
            )
            ts_eng = nc.vector if ci >= NCH - TAIL_TS_ON_DVE else nc.gpsimd
            ts_eng.tensor_scalar(
                o[sl], j[sl], half_ap[:], step_ap[:], Alu.subtract, Alu.mult
            )
            if off2 in out_cum:
                oi = out_cum.index(off2)
                prev = 0 if oi == 0 else out_cum[oi - 1]
                osl = (slice(None), slice(prev, off2))
                eng = "sync" if OUT_ENGINES is None else OUT_ENGINES[oi]
                getattr(nc, eng).dma_start(outs[osl], o[osl])

    _retag_const_memsets(nc)
    nc.compile()
    return nc


def build(bins: np.ndarray):
    key = _constants(bins)
    if key is None:
        raise NotImplementedError("unsupported bins for this kernel")
    if key not in _BUILD_CACHE:
        _BUILD_CACHE[key] = _build(*key)
    return _BUILD_CACHE[key]


def make_in_maps(Xs: np.ndarray):
    shards = Xs.reshape(NCORES, P, W)
    return [{"xs": shards[c]} for c in range(NCORES)]


def kernel(Xs: np.ndarray, bins: np.ndarray) -> np.ndarray:
    Xs = np.asarray(Xs, dtype=np.float32)
    bins = np.asarray(bins, dtype=np.float32)
    nc = build(bins)
    res = run_bass_kernel_spmd(nc, make_in_maps(Xs), core_ids=list(range(NCORES)))
    out = np.concatenate([r["out"].reshape(-1) for r in res.results])
    return out.astype(np.float32)


# revision 36
# speedup vs baseline: 1.0250x; 1.0035x over previous
"""Trainium2 Bass kernel for nn_LogOddsPerformanceTransformer.

Computes, for each element x of Xs:
    s   = log(x) - log(1-x)              (log-odds)
    idx = clip(searchsorted(bins, max(s, bins[0]), 'right') - 1, 0, NB-1)
    out = bins[idx]

bins is a uniform grid (linspace), so binning reduces to an affine floor
via the magic-number rounding trick.  The post-log chain is one fused
6-stage custom-DVE op producing the integer bin offset j, plus one
2-ALU tensor_scalar (on gpsimd) for the final affine:

    OP1:  j   = ((clip((a-b)*inv, -31, 32) + M) - M)   # M = 2^23+31
    TS:   out = (j - 0.5) * step

Clamp bounds -31/32 (instead of the exact bin edges -31.5/32.5) keep
sg + 31 >= 0 so the magic add always lands on the integer rounding grid
at 2^23; any clamp value inside the first/last bin gives the identical
bin index.

Data parallel over 8 NeuronCores; per core the 524288-element slice is
viewed as [128 x 4096].  Input DMAs (SP/HWDGE) use a ramped column-tile
grid so the activation engine is never starved; compute runs on an
independent column-chunk grid (ACT 2x Ln -> DVE fused op -> Pool TS);
output DMAs issue per compute chunk on the SP sequencer, which is idle
after the input DMAs and whose in-order semaphore waits match the chunk
completion order.
"""

import sys

sys.path.insert(0, "/opt/trn_rl_repo")

from contextlib import ExitStack

import numpy as np

import concourse.bass as bass
import concourse.tile as tile
from concourse import bacc, mybir
from concourse.bass_utils import run_bass_kernel_spmd

N = 4_194_304
NCORES = 8
NPER = N // NCORES  # 524288
P = 128
W = NPER // P  # 4096 columns per core

# --- tunables -------------------------------------------------------------
IN_TILES = (128, 256, 512, 896, 1152, 1152)  # ramped; sum = 4096
# compute grid; sum = 4096.  Chunk ends should align under tile prefix sums
# so a chunk never waits on a tile it doesn't cover.
CHUNKS = (128, 256, 512, 896, 672, 608, 480, 288, 256)
# groups of chunk indices sharing one a-pass / b-pass (Ln) activation op:
# merging late-kernel ops (whose data has long arrived) saves the ~185ns
# per-instruction activation init without hurting the pipeline ramp.
A_GROUPS = tuple((i,) for i in range(len(CHUNKS)))
B_GROUPS = tuple((i,) for i in range(len(CHUNKS)))
# out-DMA grid; boundaries must be a subset of the chunk prefix sums.
OUT_TILES = CHUNKS
# per-out issue engine names; None -> all "sync".  The second-to-last out on
# "scalar" (ACT sequencer, idle by then) overlaps the last one's SP issue.
OUT_ENGINES = ("sync",) * 7 + ("scalar", "sync")
TAIL_TS_ON_DVE = 99  # last k chunks run the final tensor_scalar on DVE not Pool
TAIL_HIPRI = 0  # unused; kept for sweep-script compat
# Chunks before the last TAIL_F32 write their result in bf16: with j (exact
# small integers, bf16-lossless) also bf16, the final tensor_scalar runs in
# the DVE 2x perf mode (2 elem/cycle).  bf16 outputs ship via gpsimd
# cast-DMAs (bf16->f32, same modeled transfer time); the rounding of the
# final value adds ~1.6e-3 norm-rel error, well inside the 2e-2 gate.  The
# last TAIL_F32 chunks stay f32 so their outs use the low-latency HWDGE
# path.  Set TAIL_F32 >= len(CHUNKS) to disable bf16 entirely.  (Measured:
# the gpsimd cast-DMA issue latency outweighs the 2x TS win -> disabled.)
TAIL_F32 = 99
# hoist the first k input DMAs ahead of the framework start barrier (they
# have no dependencies); the barrier then overlaps the first HWDGE issues.
HOIST_IN_DMAS = 2
# --------------------------------------------------------------------------

f32 = mybir.dt.float32
Alu = mybir.AluOpType
Act = mybir.ActivationFunctionType

_BUILD_CACHE: dict[tuple, object] = {}


# --- custom DVE op --------------------------------------------------------
def _j_ref(in0, in1, s0, s1, imm2):
    f = np.float32
    d = (in0.astype(f) - in1.astype(f)).astype(f)
    sg = (d * f(s0)).astype(f)
    mx = np.maximum(sg, f(imm2)).astype(f)
    lat = f(f(1.0) - f(imm2))
    mn = np.minimum(mx, lat).astype(f)
    t1 = (mn + f(s1)).astype(f)
    return (t1 - f(s1)).astype(f)


def _register_ops():
    import concourse.dve_ops as dve_ops
    from concourse.dve_spec import (
        Spec,
        Src0,
        Src1,
        C0,
        C1,
        C2,
        One,
        maxx,
        minn,
        lower,
        _has_src1,
    )
    from concourse.dve_uop import DveOpSpec

    def reg(name, spec):
        if name in dve_ops._SUB_OPCODE_FOR_NAME:
            return next(op for op in dve_ops.OPS if op.name == name)
        row = max(dve_ops._SUB_OPCODE_FOR_NAME.values()) + 1
        assert row < 0x20
        dve_ops._SUB_OPCODE_FOR_NAME[name] = row
        shas = {}
        for ver in ("v3", "v4"):
            uops = lower(spec, ver=ver)
            shas[ver] = DveOpSpec(
                name=name, opcode=row, uops=uops, rd1_en=_has_src1(spec)
            ).sha(ver)
        op = dve_ops.DveOp(name, spec, subdim=False, uops_sha=shas)
        dve_ops.OPS.append(op)
        dve_ops.CUSTOM_DVE_SPECS[name] = spec
        return op

    # j = ((clip((a-b)*C0, C2, 1-C2) + C1) - C1);  C2 = -31 so 1-C2 = 32
    sg = (Src0 - Src1) * C0
    mn = minn(maxx(sg, C2), One - C2)
    body = (mn + C1) - C1
    return reg("LOGODDS_J_ANT", Spec(body=body, reference=_j_ref))


_OP1 = _register_ops()


def _constants(bins: np.ndarray):
    """Host-side constants; returns None if the fused path can't be used
    (non-uniform bins or grid where the magic offsets aren't exact)."""
    b64 = bins.astype(np.float64)
    nb = len(bins)
    if nb != 64:
        return None
    step = np.float32((b64[-1] - b64[0]) / (nb - 1))
    inv = np.float32((nb - 1) / (b64[-1] - b64[0]))
    # sigma = s*inv ; bin edges at sigma = b0*inv + k.  Require b0*inv = -31.5
    # (true for the symmetric linspace(-6,6,64) grid) so the fixed clamp
    # bounds/magic below are exact.
    if not np.isclose(float(b64[0]) * float(inv), -31.5, atol=1e-6):
        return None
    uniform = np.allclose(
        np.diff(b64), (b64[-1] - b64[0]) / (nb - 1), rtol=0, atol=1e-5
    )
    if not uniform:
        return None
    return (float(inv), float(step))


MAGIC = float(np.float32(2.0**23 + 31.0))
SIG_LO = -31.0  # imm2 of OP1; upper clamp is 1-imm2 = 32
HALF = 0.5


def _build(inv, step):
    assert sum(IN_TILES) == W and sum(CHUNKS) == W and sum(OUT_TILES) == W
    ccum = np.cumsum(CHUNKS)
    assert set(np.cumsum(OUT_TILES)) <= set(ccum), "OUT_TILES must nest in CHUNKS"

    return _build_body(inv, step)


def _retag_const_memsets(nc):
    """Strip the framework preamble's const-AP memsets when nothing in the
    kernel references those const tensors (this kernel passes all activation
    biases and tensor_scalar operands as its own APs or immediates).  The
    memsets otherwise gate the kernel start barrier by ~0.3-0.5us.  If any
    instruction does reference a const AP, fall back to retagging the memsets
    from Pool (95ns Q7 launch each) to the cheaper DVE engine."""
    fn = nc.m.functions[0]
    referenced = any(
        "memref='const" in str(arg)
        for bb in fn.blocks
        for ins in bb.instructions
        for arg in (getattr(ins, "ins", None) or [])
    )
    for bb in fn.blocks:
        dead = [
            ins
            for ins in list(bb.instructions)
            if type(ins).__name__ == "InstMemset"
            and "memref='const" in str(ins.outs[0])
        ]
        for ins in dead:
            if referenced:
                if ins.engine == mybir.EngineType.Pool:
                    ins.engine = mybir.EngineType.DVE
            else:
                bb.instructions.remove(ins)


def _build_body(inv, step):
    nc = bacc.Bacc("TRN2", target_bir_lowering=False, debug=False)
    xs = nc.dram_tensor("xs", [P, W], f32, kind="ExternalInput").ap()
    outs = nc.dram_tensor("out", [P, W], f32, kind="ExternalOutput").ap()

    with tile.TileContext(nc) as tc, ExitStack() as ctx:
        tmp = ctx.enter_context(tc.tile_pool(name="tmp", bufs=1))

        bf16 = mybir.dt.bfloat16
        x = tmp.tile([P, W], f32, tag="x")
        a = tmp.tile([P, W], f32, tag="a")
        b = tmp.tile([P, W], f32, tag="b")
        j = tmp.tile([P, W], bf16, tag="j")
        o = tmp.tile([P, W], f32, tag="o")
        ob = tmp.tile([P, W], bf16, tag="ob")

        # all input DMAs issued first (high priority) so the out DMAs never
        # starve later input tiles
        with tc.high_priority():
            off = 0
            for w in IN_TILES:
                sl = (slice(None), slice(off, off + w))
                nc.sync.dma_start(x[sl], xs[sl])
                off += w

        # scalar constants built with DVE memsets (idle engine) so no
        # const-pool Memset gates the start barrier
        bias0 = tmp.tile([P, 1], f32, tag="bias0")
        bias1 = tmp.tile([P, 1], f32, tag="bias1")
        half_ap = tmp.tile([P, 1], f32, tag="half")
        step_ap = tmp.tile([P, 1], f32, tag="step")
        nc.vector.memset(bias0[:], 0.0)
        nc.vector.memset(bias1[:], 1.0)
        nc.vector.memset(half_ap[:], HALF)
        nc.vector.memset(step_ap[:], step)
        # warmup: forces the Ln act-table load to run during the DMA ramp
        # instead of gating the first real activation
        warm = tmp.tile([P, 1], f32, tag="warm")
        nc.scalar.activation(warm[:], bias1[:], Act.Ln, bias0[:])

        NCH = len(CHUNKS)
        ccum = [0] + list(np.cumsum(CHUNKS))
        assert sorted(i for g in A_GROUPS for i in g) == list(range(NCH))
        assert sorted(i for g in B_GROUPS for i in g) == list(range(NCH))
        a_first = {g[0]: g for g in A_GROUPS}
        b_first = {g[0]: g for g in B_GROUPS}
        out_cum = list(np.cumsum(OUT_TILES))
        for ci in range(NCH):
            if ci in a_first:
                g = a_first[ci]
                gsl = (slice(None), slice(ccum[g[0]], ccum[g[-1] + 1]))
                nc.scalar.activation(a[gsl], x[gsl], Act.Ln, bias0[:])
            if ci in b_first:
                g = b_first[ci]
                gsl = (slice(None), slice(ccum[g[0]], ccum[g[-1] + 1]))
                nc.scalar.activation(b[gsl], x[gsl], Act.Ln, bias1[:], -1.0)
            off, off2 = ccum[ci], ccum[ci + 1]
            sl = (slice(None), slice(off, off2))
            is_bf16 = ci < NCH - TAIL_F32
            nc.vector._custom_dve(
                _OP1, out=j[sl], in0=a[sl], in1=b[sl], s0=inv, s1=MAGIC, imm2=SIG_LO
            )
            ts_eng = nc.vector if ci >= NCH - TAIL_TS_ON_DVE else nc.gpsimd
            ts_eng.tensor_scalar(
                (ob if is_bf16 else o)[sl],
                j[sl],
                half_ap[:],
                step_ap[:],
                Alu.subtract,
                Alu.mult,
            )
            if off2 in out_cum:
                oi = out_cum.index(off2)
                prev = 0 if oi == 0 else out_cum[oi - 1]
                osl = (slice(None), slice(prev, off2))
                if is_bf16:
                    nc.gpsimd.dma_start(outs[osl], ob[osl])
                else:
                    eng = "sync" if OUT_ENGINES is None else OUT_ENGINES[oi]
                    getattr(nc, eng).dma_start(outs[osl], o[osl])

    _retag_const_memsets(nc)
    _hoist_in_dmas(nc)
    nc.compile()
    return nc


def _hoist_in_dmas(nc):
    """Move the first HOIST_IN_DMAS input DMACopy instructions (SP engine,
    no semaphore waits) to the front of the first block, ahead of the
    framework's start-barrier drains.  Their HWDGE issue then overlaps the
    barrier instead of waiting for it, pulling the whole pipeline earlier.
    Per-engine program order is preserved (they were SP's first body
    instructions)."""
    if HOIST_IN_DMAS <= 0:
        return
    blocks = list(nc.m.functions[0].blocks)
    if len(blocks) < 2:
        return
    b0, body = blocks[0], blocks[1]
    moved = []
    for ins in list(body.instructions):
        if (
            type(ins).__name__ == "InstDMACopy"
            and ins.engine == mybir.EngineType.SP
        ):
            si = ins.sync_info
            if si is not None and si.on_wait:
                break
            moved.append(ins)
            if len(moved) >= HOIST_IN_DMAS:
                break
    for ins in moved:
        body.instructions.remove(ins)
    # position 1: after the leading dummy InstCall, before the barrier drains
    for k, ins in enumerate(moved):
        b0.instructions.insert(1 + k, ins)


def build(bins: np.ndarray):
    key = _constants(bins)
    if key is None:
        raise NotImplementedError("unsupported bins for this kernel")
    if key not in _BUILD_CACHE:
        _BUILD_CACHE[key] = _build(*key)
    return _BUILD_CACHE[key]


def make_in_maps(Xs: np.ndarray):
    shards = Xs.reshape(NCORES, P, W)
    return [{"xs": shards[c]} for c in range(NCORES)]


def kernel(Xs: np.ndarray, bins: np.ndarray) -> np.ndarray:
    Xs = np.asarray(Xs, dtype=np.float32)
    bins = np.asarray(bins, dtype=np.float32)
    nc = build(bins)
    res = run_bass_kernel_spmd(nc, make_in_maps(Xs), core_ids=list(range(NCORES)))
    out = np.concatenate([r["out"].reshape(-1) for r in res.results])
    return out.astype(np.float32)


# revision 41
# speedup vs baseline: 1.0351x; 1.0098x over previous
"""Trainium2 Bass kernel for nn_LogOddsPerformanceTransformer.

Computes, for each element x of Xs:
    s   = log(x) - log(1-x)              (log-odds)
    idx = clip(searchsorted(bins, max(s, bins[0]), 'right') - 1, 0, NB-1)
    out = bins[idx]

bins is a uniform grid (linspace), so binning reduces to an affine floor
via the magic-number rounding trick.  The post-log chain is one fused
6-stage custom-DVE op producing the integer bin offset j, plus one
2-ALU tensor_scalar (on gpsimd) for the final affine:

    OP1:  j   = ((clip((a-b)*inv, -31, 32) + M) - M)   # M = 2^23+31
    TS:   out = (j - 0.5) * step

Clamp bounds -31/32 (instead of the exact bin edges -31.5/32.5) keep
sg + 31 >= 0 so the magic add always lands on the integer rounding grid
at 2^23; any clamp value inside the first/last bin gives the identical
bin index.

Data parallel over 8 NeuronCores; per core the 524288-element slice is
viewed as [128 x 4096].  Input DMAs (SP/HWDGE) use a ramped column-tile
grid so the activation engine is never starved; compute runs on an
independent column-chunk grid (ACT 2x Ln -> DVE fused op -> Pool TS);
output DMAs issue per compute chunk on the SP sequencer, which is idle
after the input DMAs and whose in-order semaphore waits match the chunk
completion order.
"""

import sys

sys.path.insert(0, "/opt/trn_rl_repo")

from contextlib import ExitStack

import numpy as np

import concourse.bass as bass
import concourse.tile as tile
from concourse import bacc, mybir
from concourse.bass_utils import run_bass_kernel_spmd

N = 4_194_304
NCORES = 8
NPER = N // NCORES  # 524288
P = 128
W = NPER // P  # 4096 columns per core

# --- tunables -------------------------------------------------------------
IN_TILES = (128, 256, 512, 896, 1152, 1152)  # ramped; sum = 4096
# compute grid; sum = 4096.  Chunk ends should align under tile prefix sums
# so a chunk never waits on a tile it doesn't cover.
CHUNKS = (128, 256, 512, 896, 672, 608, 480, 288, 256)
# groups of chunk indices sharing one a-pass / b-pass (Ln) activation op:
# merging late-kernel ops (whose data has long arrived) saves the ~185ns
# per-instruction activation init without hurting the pipeline ramp.
A_GROUPS = tuple((i,) for i in range(len(CHUNKS)))
B_GROUPS = tuple((i,) for i in range(len(CHUNKS)))
# out-DMA grid; boundaries must be a subset of the chunk prefix sums.
OUT_TILES = CHUNKS
# per-out issue engine names; None -> all "sync".  The second-to-last out on
# "scalar" (ACT sequencer, idle by then) overlaps the last one's SP issue.
OUT_ENGINES = ("sync",) * 7 + ("scalar", "sync")
TAIL_TS_ON_DVE = 99  # last k chunks run the final tensor_scalar on DVE not Pool
TAIL_HIPRI = 0  # unused; kept for sweep-script compat
# Chunks before the last TAIL_F32 write their result in bf16: with j (exact
# small integers, bf16-lossless) also bf16, the final tensor_scalar runs in
# the DVE 2x perf mode (2 elem/cycle).  bf16 outputs ship via gpsimd
# cast-DMAs (bf16->f32, same modeled transfer time); the rounding of the
# final value adds ~1.6e-3 norm-rel error, well inside the 2e-2 gate.  The
# last TAIL_F32 chunks stay f32 so their outs use the low-latency HWDGE
# path.  Set TAIL_F32 >= len(CHUNKS) to disable bf16 entirely.  (Measured:
# the gpsimd cast-DMA issue latency outweighs the 2x TS win -> disabled.)
TAIL_F32 = 99
# hoist the first k input DMAs ahead of the framework start barrier (they
# have no dependencies); the barrier then overlaps the first HWDGE issues.
HOIST_IN_DMAS = 4
HOIST_TABLE = True  # post-compile: move the act-table load pre-barrier
WARMUP = False  # warmup activation unneeded once the table load is hoisted
# --------------------------------------------------------------------------

f32 = mybir.dt.float32
Alu = mybir.AluOpType
Act = mybir.ActivationFunctionType

_BUILD_CACHE: dict[tuple, object] = {}


# --- custom DVE op --------------------------------------------------------
def _j_ref(in0, in1, s0, s1, imm2):
    f = np.float32
    d = (in0.astype(f) - in1.astype(f)).astype(f)
    sg = (d * f(s0)).astype(f)
    mx = np.maximum(sg, f(imm2)).astype(f)
    lat = f(f(1.0) - f(imm2))
    mn = np.minimum(mx, lat).astype(f)
    t1 = (mn + f(s1)).astype(f)
    return (t1 - f(s1)).astype(f)


def _register_ops():
    import concourse.dve_ops as dve_ops
    from concourse.dve_spec import (
        Spec,
        Src0,
        Src1,
        C0,
        C1,
        C2,
        One,
        maxx,
        minn,
        lower,
        _has_src1,
    )
    from concourse.dve_uop import DveOpSpec

    def reg(name, spec):
        if name in dve_ops._SUB_OPCODE_FOR_NAME:
            return next(op for op in dve_ops.OPS if op.name == name)
        row = max(dve_ops._SUB_OPCODE_FOR_NAME.values()) + 1
        assert row < 0x20
        dve_ops._SUB_OPCODE_FOR_NAME[name] = row
        shas = {}
        for ver in ("v3", "v4"):
            uops = lower(spec, ver=ver)
            shas[ver] = DveOpSpec(
                name=name, opcode=row, uops=uops, rd1_en=_has_src1(spec)
            ).sha(ver)
        op = dve_ops.DveOp(name, spec, subdim=False, uops_sha=shas)
        dve_ops.OPS.append(op)
        dve_ops.CUSTOM_DVE_SPECS[name] = spec
        return op

    # j = ((clip((a-b)*C0, C2, 1-C2) + C1) - C1);  C2 = -31 so 1-C2 = 32
    sg = (Src0 - Src1) * C0
    mn = minn(maxx(sg, C2), One - C2)
    body = (mn + C1) - C1
    return reg("LOGODDS_J_ANT", Spec(body=body, reference=_j_ref))


_OP1 = _register_ops()


def _constants(bins: np.ndarray):
    """Host-side constants; returns None if the fused path can't be used
    (non-uniform bins or grid where the magic offsets aren't exact)."""
    b64 = bins.astype(np.float64)
    nb = len(bins)
    if nb != 64:
        return None
    step = np.float32((b64[-1] - b64[0]) / (nb - 1))
    inv = np.float32((nb - 1) / (b64[-1] - b64[0]))
    # sigma = s*inv ; bin edges at sigma = b0*inv + k.  Require b0*inv = -31.5
    # (true for the symmetric linspace(-6,6,64) grid) so the fixed clamp
    # bounds/magic below are exact.
    if not np.isclose(float(b64[0]) * float(inv), -31.5, atol=1e-6):
        return None
    uniform = np.allclose(
        np.diff(b64), (b64[-1] - b64[0]) / (nb - 1), rtol=0, atol=1e-5
    )
    if not uniform:
        return None
    return (float(inv), float(step))


MAGIC = float(np.float32(2.0**23 + 31.0))
SIG_LO = -31.0  # imm2 of OP1; upper clamp is 1-imm2 = 32
HALF = 0.5


def _build(inv, step):
    assert sum(IN_TILES) == W and sum(CHUNKS) == W and sum(OUT_TILES) == W
    ccum = np.cumsum(CHUNKS)
    assert set(np.cumsum(OUT_TILES)) <= set(ccum), "OUT_TILES must nest in CHUNKS"

    return _build_body(inv, step)


def _retag_const_memsets(nc):
    """Strip the framework preamble's const-AP memsets when nothing in the
    kernel references those const tensors (this kernel passes all activation
    biases and tensor_scalar operands as its own APs or immediates).  The
    memsets otherwise gate the kernel start barrier by ~0.3-0.5us.  If any
    instruction does reference a const AP, fall back to retagging the memsets
    from Pool (95ns Q7 launch each) to the cheaper DVE engine."""
    fn = nc.m.functions[0]
    referenced = any(
        "memref='const" in str(arg)
        for bb in fn.blocks
        for ins in bb.instructions
        for arg in (getattr(ins, "ins", None) or [])
    )
    for bb in fn.blocks:
        dead = [
            ins
            for ins in list(bb.instructions)
            if type(ins).__name__ == "InstMemset"
            and "memref='const" in str(ins.outs[0])
        ]
        for ins in dead:
            if referenced:
                if ins.engine == mybir.EngineType.Pool:
                    ins.engine = mybir.EngineType.DVE
            else:
                bb.instructions.remove(ins)


def _build_body(inv, step):
    nc = bacc.Bacc("TRN2", target_bir_lowering=False, debug=False)
    xs = nc.dram_tensor("xs", [P, W], f32, kind="ExternalInput").ap()
    outs = nc.dram_tensor("out", [P, W], f32, kind="ExternalOutput").ap()

    with tile.TileContext(nc) as tc, ExitStack() as ctx:
        tmp = ctx.enter_context(tc.tile_pool(name="tmp", bufs=1))

        bf16 = mybir.dt.bfloat16
        x = tmp.tile([P, W], f32, tag="x")
        a = tmp.tile([P, W], f32, tag="a")
        b = tmp.tile([P, W], f32, tag="b")
        j = tmp.tile([P, W], bf16, tag="j")
        o = tmp.tile([P, W], f32, tag="o")
        ob = tmp.tile([P, W], bf16, tag="ob")

        # all input DMAs issued first (high priority) so the out DMAs never
        # starve later input tiles
        with tc.high_priority():
            off = 0
            for w in IN_TILES:
                sl = (slice(None), slice(off, off + w))
                nc.sync.dma_start(x[sl], xs[sl])
                off += w

        # scalar constants built with DVE memsets (idle engine) so no
        # const-pool Memset gates the start barrier
        bias0 = tmp.tile([P, 1], f32, tag="bias0")
        bias1 = tmp.tile([P, 1], f32, tag="bias1")
        half_ap = tmp.tile([P, 1], f32, tag="half")
        step_ap = tmp.tile([P, 1], f32, tag="step")
        nc.vector.memset(bias0[:], 0.0)
        nc.vector.memset(bias1[:], 1.0)
        nc.vector.memset(half_ap[:], HALF)
        nc.vector.memset(step_ap[:], step)
        if WARMUP:
            # warmup: forces the Ln act-table load to run during the DMA
            # ramp instead of gating the first real activation
            warm = tmp.tile([P, 1], f32, tag="warm")
            nc.scalar.activation(warm[:], bias1[:], Act.Ln, bias0[:])

        NCH = len(CHUNKS)
        ccum = [0] + list(np.cumsum(CHUNKS))
        assert sorted(i for g in A_GROUPS for i in g) == list(range(NCH))
        assert sorted(i for g in B_GROUPS for i in g) == list(range(NCH))
        a_first = {g[0]: g for g in A_GROUPS}
        b_first = {g[0]: g for g in B_GROUPS}
        out_cum = list(np.cumsum(OUT_TILES))
        for ci in range(NCH):
            if ci in a_first:
                g = a_first[ci]
                gsl = (slice(None), slice(ccum[g[0]], ccum[g[-1] + 1]))
                nc.scalar.activation(a[gsl], x[gsl], Act.Ln, bias0[:])
            if ci in b_first:
                g = b_first[ci]
                gsl = (slice(None), slice(ccum[g[0]], ccum[g[-1] + 1]))
                nc.scalar.activation(b[gsl], x[gsl], Act.Ln, bias1[:], -1.0)
            off, off2 = ccum[ci], ccum[ci + 1]
            sl = (slice(None), slice(off, off2))
            is_bf16 = ci < NCH - TAIL_F32
            nc.vector._custom_dve(
                _OP1, out=j[sl], in0=a[sl], in1=b[sl], s0=inv, s1=MAGIC, imm2=SIG_LO
            )
            ts_eng = nc.vector if ci >= NCH - TAIL_TS_ON_DVE else nc.gpsimd
            ts_eng.tensor_scalar(
                (ob if is_bf16 else o)[sl],
                j[sl],
                half_ap[:],
                step_ap[:],
                Alu.subtract,
                Alu.mult,
            )
            if off2 in out_cum:
                oi = out_cum.index(off2)
                prev = 0 if oi == 0 else out_cum[oi - 1]
                osl = (slice(None), slice(prev, off2))
                if is_bf16:
                    nc.gpsimd.dma_start(outs[osl], ob[osl])
                else:
                    eng = "sync" if OUT_ENGINES is None else OUT_ENGINES[oi]
                    getattr(nc, eng).dma_start(outs[osl], o[osl])

    _retag_const_memsets(nc)
    _hoist_in_dmas(nc)
    nc.compile()
    if HOIST_TABLE:
        _hoist_act_table_load(nc)
    return nc


def _hoist_act_table_load(nc):
    """Post-compile: move the (dependency-free) Ln act-table load ahead of
    the start-barrier drains so its 1.3us never gates the first
    data-dependent activation.  It must run post-compile because the BIR
    lowering inserts InstLoadActFuncSet lazily before the first
    activation."""
    blocks = list(nc.m.functions[0].blocks)
    if len(blocks) < 2:
        return
    b0, body = blocks[0], blocks[1]
    for ins in list(body.instructions):
        if type(ins).__name__ == "InstLoadActFuncSet":
            si = ins.sync_info
            if si is None or not si.on_wait:
                body.instructions.remove(ins)
                b0.instructions.insert(1, ins)
            return


def _hoist_in_dmas(nc):
    """Move the first HOIST_IN_DMAS input DMACopy instructions (SP engine,
    no semaphore waits) to the front of the first block, ahead of the
    framework's start-barrier drains.  Their HWDGE issue then overlaps the
    barrier instead of waiting for it, pulling the whole pipeline earlier.
    Per-engine program order is preserved (they were SP's first body
    instructions)."""
    if HOIST_IN_DMAS <= 0:
        return
    blocks = list(nc.m.functions[0].blocks)
    if len(blocks) < 2:
        return
    b0, body = blocks[0], blocks[1]
    moved = []
    for ins in list(body.instructions):
        if (
            type(ins).__name__ == "InstDMACopy"
            and ins.engine == mybir.EngineType.SP
        ):
            si = ins.sync_info
            if si is not None and si.on_wait:
                break
            moved.append(ins)
            if len(moved) >= HOIST_IN_DMAS:
                break
    # the Ln act-table load has no dependencies; pre-barrier it never gates
    # the first data-dependent activation
    for ins in list(body.instructions):
        if type(ins).__name__ == "InstLoadActFuncSet":
            si = ins.sync_info
            if si is None or not si.on_wait:
                moved.insert(0, ins)
            break
    for ins in moved:
        body.instructions.remove(ins)
    # position 1: after the leading dummy InstCall, before the barrier drains
    for k, ins in enumerate(moved):
        b0.instructions.insert(1 + k, ins)


def build(bins: np.ndarray):
    key = _constants(bins)
    if key is None:
        raise NotImplementedError("unsupported bins for this kernel")
    if key not in _BUILD_CACHE:
        _BUILD_CACHE[key] = _build(*key)
    return _BUILD_CACHE[key]


def make_in_maps(Xs: np.ndarray):
    shards = Xs.reshape(NCORES, P, W)
    return [{"xs": shards[c]} for c in range(NCORES)]


def kernel(Xs: np.ndarray, bins: np.ndarray) -> np.ndarray:
    Xs = np.asarray(Xs, dtype=np.float32)
    bins = np.asarray(bins, dtype=np.float32)
    nc = build(bins)
    res = run_bass_kernel_spmd(nc, make_in_maps(Xs), core_ids=list(range(NCORES)))
    out = np.concatenate([r["out"].reshape(-1) for r in res.results])
    return out.astype(np.float32)


# revision 43
# speedup vs baseline: 1.0510x; 1.0154x over previous
"""Trainium2 Bass kernel for nn_LogOddsPerformanceTransformer.

Computes, for each element x of Xs:
    s   = log(x) - log(1-x)              (log-odds)
    idx = clip(searchsorted(bins, max(s, bins[0]), 'right') - 1, 0, NB-1)
    out = bins[idx]

bins is a uniform grid (linspace), so binning reduces to an affine floor
via the magic-number rounding trick.  The post-log chain is one fused
6-stage custom-DVE op producing the integer bin offset j, plus one
2-ALU tensor_scalar (on gpsimd) for the final affine:

    OP1:  j   = ((clip((a-b)*inv, -31, 32) + M) - M)   # M = 2^23+31
    TS:   out = (j - 0.5) * step

Clamp bounds -31/32 (instead of the exact bin edges -31.5/32.5) keep
sg + 31 >= 0 so the magic add always lands on the integer rounding grid
at 2^23; any clamp value inside the first/last bin gives the identical
bin index.

Data parallel over 8 NeuronCores; per core the 524288-element slice is
viewed as [128 x 4096].  Input DMAs (SP/HWDGE) use a ramped column-tile
grid so the activation engine is never starved; compute runs on an
independent column-chunk grid (ACT 2x Ln -> DVE fused op -> Pool TS);
output DMAs issue per compute chunk on the SP sequencer, which is idle
after the input DMAs and whose in-order semaphore waits match the chunk
completion order.
"""

import sys

sys.path.insert(0, "/opt/trn_rl_repo")

from contextlib import ExitStack

import numpy as np

import concourse.bass as bass
import concourse.tile as tile
from concourse import bacc, mybir
from concourse.bass_utils import run_bass_kernel_spmd

N = 4_194_304
NCORES = 8
NPER = N // NCORES  # 524288
P = 128
W = NPER // P  # 4096 columns per core

# --- tunables -------------------------------------------------------------
IN_TILES = (128, 256, 512, 896, 1152, 1152)  # ramped; sum = 4096
# compute grid; sum = 4096.  Chunk ends should align under tile prefix sums
# so a chunk never waits on a tile it doesn't cover.
CHUNKS = (128, 256, 512, 896, 672, 608, 480, 288, 256)
# groups of chunk indices sharing one a-pass / b-pass (Ln) activation op:
# merging late-kernel ops (whose data has long arrived) saves the ~185ns
# per-instruction activation init without hurting the pipeline ramp.
A_GROUPS = tuple((i,) for i in range(len(CHUNKS)))
B_GROUPS = tuple((i,) for i in range(len(CHUNKS)))
# out-DMA grid; boundaries must be a subset of the chunk prefix sums.
OUT_TILES = CHUNKS
# per-out issue engine names; None -> all "sync".  The second-to-last out on
# "scalar" (ACT sequencer, idle by then) overlaps the last one's SP issue.
OUT_ENGINES = ("sync",) * 7 + ("scalar", "sync")
TAIL_TS_ON_DVE = 99  # last k chunks run the final tensor_scalar on DVE not Pool
TAIL_HIPRI = 0  # unused; kept for sweep-script compat
# Chunks before the last TAIL_F32 write their result in bf16: with j (exact
# small integers, bf16-lossless) also bf16, the final tensor_scalar runs in
# the DVE 2x perf mode (2 elem/cycle).  bf16 outputs ship via gpsimd
# cast-DMAs (bf16->f32, same modeled transfer time); the rounding of the
# final value adds ~1.6e-3 norm-rel error, well inside the 2e-2 gate.  The
# last TAIL_F32 chunks stay f32 so their outs use the low-latency HWDGE
# path.  Set TAIL_F32 >= len(CHUNKS) to disable bf16 entirely.  (Measured:
# the gpsimd cast-DMA issue latency outweighs the 2x TS win -> disabled.)
TAIL_F32 = 99
# hoist the first k input DMAs ahead of the framework start barrier (they
# have no dependencies); the barrier then overlaps the first HWDGE issues.
HOIST_IN_DMAS = 4
HOIST_TABLE = True  # post-compile: move the act-table load pre-barrier
WARMUP = False  # warmup activation unneeded once the table load is hoisted
STRIP_EXIT_BARRIER = True  # drop the redundant second exit barrier round
# --------------------------------------------------------------------------

f32 = mybir.dt.float32
Alu = mybir.AluOpType
Act = mybir.ActivationFunctionType

_BUILD_CACHE: dict[tuple, object] = {}


# --- custom DVE op --------------------------------------------------------
def _j_ref(in0, in1, s0, s1, imm2):
    f = np.float32
    d = (in0.astype(f) - in1.astype(f)).astype(f)
    sg = (d * f(s0)).astype(f)
    mx = np.maximum(sg, f(imm2)).astype(f)
    lat = f(f(1.0) - f(imm2))
    mn = np.minimum(mx, lat).astype(f)
    t1 = (mn + f(s1)).astype(f)
    return (t1 - f(s1)).astype(f)


def _register_ops():
    import concourse.dve_ops as dve_ops
    from concourse.dve_spec import (
        Spec,
        Src0,
        Src1,
        C0,
        C1,
        C2,
        One,
        maxx,
        minn,
        lower,
        _has_src1,
    )
    from concourse.dve_uop import DveOpSpec

    def reg(name, spec):
        if name in dve_ops._SUB_OPCODE_FOR_NAME:
            return next(op for op in dve_ops.OPS if op.name == name)
        row = max(dve_ops._SUB_OPCODE_FOR_NAME.values()) + 1
        assert row < 0x20
        dve_ops._SUB_OPCODE_FOR_NAME[name] = row
        shas = {}
        for ver in ("v3", "v4"):
            uops = lower(spec, ver=ver)
            shas[ver] = DveOpSpec(
                name=name, opcode=row, uops=uops, rd1_en=_has_src1(spec)
            ).sha(ver)
        op = dve_ops.DveOp(name, spec, subdim=False, uops_sha=shas)
        dve_ops.OPS.append(op)
        dve_ops.CUSTOM_DVE_SPECS[name] = spec
        return op

    # j = ((clip((a-b)*C0, C2, 1-C2) + C1) - C1);  C2 = -31 so 1-C2 = 32
    sg = (Src0 - Src1) * C0
    mn = minn(maxx(sg, C2), One - C2)
    body = (mn + C1) - C1
    return reg("LOGODDS_J_ANT", Spec(body=body, reference=_j_ref))


_OP1 = _register_ops()


def _constants(bins: np.ndarray):
    """Host-side constants; returns None if the fused path can't be used
    (non-uniform bins or grid where the magic offsets aren't exact)."""
    b64 = bins.astype(np.float64)
    nb = len(bins)
    if nb != 64:
        return None
    step = np.float32((b64[-1] - b64[0]) / (nb - 1))
    inv = np.float32((nb - 1) / (b64[-1] - b64[0]))
    # sigma = s*inv ; bin edges at sigma = b0*inv + k.  Require b0*inv = -31.5
    # (true for the symmetric linspace(-6,6,64) grid) so the fixed clamp
    # bounds/magic below are exact.
    if not np.isclose(float(b64[0]) * float(inv), -31.5, atol=1e-6):
        return None
    uniform = np.allclose(
        np.diff(b64), (b64[-1] - b64[0]) / (nb - 1), rtol=0, atol=1e-5
    )
    if not uniform:
        return None
    return (float(inv), float(step))


MAGIC = float(np.float32(2.0**23 + 31.0))
SIG_LO = -31.0  # imm2 of OP1; upper clamp is 1-imm2 = 32
HALF = 0.5


def _build(inv, step):
    assert sum(IN_TILES) == W and sum(CHUNKS) == W and sum(OUT_TILES) == W
    ccum = np.cumsum(CHUNKS)
    assert set(np.cumsum(OUT_TILES)) <= set(ccum), "OUT_TILES must nest in CHUNKS"

    return _build_body(inv, step)


def _retag_const_memsets(nc):
    """Strip the framework preamble's const-AP memsets when nothing in the
    kernel references those const tensors (this kernel passes all activation
    biases and tensor_scalar operands as its own APs or immediates).  The
    memsets otherwise gate the kernel start barrier by ~0.3-0.5us.  If any
    instruction does reference a const AP, fall back to retagging the memsets
    from Pool (95ns Q7 launch each) to the cheaper DVE engine."""
    fn = nc.m.functions[0]
    referenced = any(
        "memref='const" in str(arg)
        for bb in fn.blocks
        for ins in bb.instructions
        for arg in (getattr(ins, "ins", None) or [])
    )
    for bb in fn.blocks:
        dead = [
            ins
            for ins in list(bb.instructions)
            if type(ins).__name__ == "InstMemset"
            and "memref='const" in str(ins.outs[0])
        ]
        for ins in dead:
            if referenced:
                if ins.engine == mybir.EngineType.Pool:
                    ins.engine = mybir.EngineType.DVE
            else:
                bb.instructions.remove(ins)


def _build_body(inv, step):
    nc = bacc.Bacc("TRN2", target_bir_lowering=False, debug=False)
    xs = nc.dram_tensor("xs", [P, W], f32, kind="ExternalInput").ap()
    outs = nc.dram_tensor("out", [P, W], f32, kind="ExternalOutput").ap()

    with tile.TileContext(nc) as tc, ExitStack() as ctx:
        tmp = ctx.enter_context(tc.tile_pool(name="tmp", bufs=1))

        bf16 = mybir.dt.bfloat16
        x = tmp.tile([P, W], f32, tag="x")
        a = tmp.tile([P, W], f32, tag="a")
        b = tmp.tile([P, W], f32, tag="b")
        j = tmp.tile([P, W], bf16, tag="j")
        o = tmp.tile([P, W], f32, tag="o")
        ob = tmp.tile([P, W], bf16, tag="ob")

        # all input DMAs issued first (high priority) so the out DMAs never
        # starve later input tiles
        with tc.high_priority():
            off = 0
            for w in IN_TILES:
                sl = (slice(None), slice(off, off + w))
                nc.sync.dma_start(x[sl], xs[sl])
                off += w

        # scalar constants built with DVE memsets (idle engine) so no
        # const-pool Memset gates the start barrier
        bias0 = tmp.tile([P, 1], f32, tag="bias0")
        bias1 = tmp.tile([P, 1], f32, tag="bias1")
        half_ap = tmp.tile([P, 1], f32, tag="half")
        step_ap = tmp.tile([P, 1], f32, tag="step")
        nc.vector.memset(bias0[:], 0.0)
        nc.vector.memset(bias1[:], 1.0)
        nc.vector.memset(half_ap[:], HALF)
        nc.vector.memset(step_ap[:], step)
        if WARMUP:
            # warmup: forces the Ln act-table load to run during the DMA
            # ramp instead of gating the first real activation
            warm = tmp.tile([P, 1], f32, tag="warm")
            nc.scalar.activation(warm[:], bias1[:], Act.Ln, bias0[:])

        NCH = len(CHUNKS)
        ccum = [0] + list(np.cumsum(CHUNKS))
        assert sorted(i for g in A_GROUPS for i in g) == list(range(NCH))
        assert sorted(i for g in B_GROUPS for i in g) == list(range(NCH))
        a_first = {g[0]: g for g in A_GROUPS}
        b_first = {g[0]: g for g in B_GROUPS}
        out_cum = list(np.cumsum(OUT_TILES))
        for ci in range(NCH):
            if ci in a_first:
                g = a_first[ci]
                gsl = (slice(None), slice(ccum[g[0]], ccum[g[-1] + 1]))
                nc.scalar.activation(a[gsl], x[gsl], Act.Ln, bias0[:])
            if ci in b_first:
                g = b_first[ci]
                gsl = (slice(None), slice(ccum[g[0]], ccum[g[-1] + 1]))
                nc.scalar.activation(b[gsl], x[gsl], Act.Ln, bias1[:], -1.0)
            off, off2 = ccum[ci], ccum[ci + 1]
            sl = (slice(None), slice(off, off2))
            is_bf16 = ci < NCH - TAIL_F32
            nc.vector._custom_dve(
                _OP1, out=j[sl], in0=a[sl], in1=b[sl], s0=inv, s1=MAGIC, imm2=SIG_LO
            )
            ts_eng = nc.vector if ci >= NCH - TAIL_TS_ON_DVE else nc.gpsimd
            ts_eng.tensor_scalar(
                (ob if is_bf16 else o)[sl],
                j[sl],
                half_ap[:],
                step_ap[:],
                Alu.subtract,
                Alu.mult,
            )
            if off2 in out_cum:
                oi = out_cum.index(off2)
                prev = 0 if oi == 0 else out_cum[oi - 1]
                osl = (slice(None), slice(prev, off2))
                if is_bf16:
                    nc.gpsimd.dma_start(outs[osl], ob[osl])
                else:
                    eng = "sync" if OUT_ENGINES is None else OUT_ENGINES[oi]
                    getattr(nc, eng).dma_start(outs[osl], o[osl])

    _retag_const_memsets(nc)
    _hoist_in_dmas(nc)
    nc.compile()
    if HOIST_TABLE:
        _hoist_act_table_load(nc)
    if STRIP_EXIT_BARRIER:
        _strip_second_exit_barrier(nc)
    return nc


def _strip_second_exit_barrier(nc):
    """The compiled epilogue runs two all-engine barrier rounds separated by
    the Pool dma_reset ISA op.  The second round only re-synchronizes the
    engines after that reset; every engine's stream may simply end at round
    one (the runtime waits for each queue to drain independently), so drop
    round two (~0.3us off the kernel end)."""
    blocks = list(nc.m.functions[0].blocks)
    epi = blocks[-1]
    ins_list = list(epi.instructions)
    # find the Pool ISA (dma_reset) marker; everything after it that is a
    # Drain/EventSemaphore barrier pair is round two
    isa_idx = None
    for i, ins in enumerate(ins_list):
        if type(ins).__name__ == "InstISA" and ins.engine == mybir.EngineType.Pool:
            isa_idx = i
    if isa_idx is None:
        return
    tail = ins_list[isa_idx + 1 :]
    if tail and all(
        type(t).__name__ in ("InstDrain", "InstEventSemaphore") for t in tail
    ):
        for t in tail:
            epi.instructions.remove(t)


def _hoist_act_table_load(nc):
    """Post-compile: move the (dependency-free) Ln act-table load ahead of
    the start-barrier drains so its 1.3us never gates the first
    data-dependent activation.  It must run post-compile because the BIR
    lowering inserts InstLoadActFuncSet lazily before the first
    activation."""
    blocks = list(nc.m.functions[0].blocks)
    if len(blocks) < 2:
        return
    b0, body = blocks[0], blocks[1]
    for ins in list(body.instructions):
        if type(ins).__name__ == "InstLoadActFuncSet":
            si = ins.sync_info
            if si is None or not si.on_wait:
                body.instructions.remove(ins)
                b0.instructions.insert(1, ins)
            return


def _hoist_in_dmas(nc):
    """Move the first HOIST_IN_DMAS input DMACopy instructions (SP engine,
    no semaphore waits) to the front of the first block, ahead of the
    framework's start-barrier drains.  Their HWDGE issue then overlaps the
    barrier instead of waiting for it, pulling the whole pipeline earlier.
    Per-engine program order is preserved (they were SP's first body
    instructions)."""
    if HOIST_IN_DMAS <= 0:
        return
    blocks = list(nc.m.functions[0].blocks)
    if len(blocks) < 2:
        return
    b0, body = blocks[0], blocks[1]
    moved = []
    for ins in list(body.instructions):
        if (
            type(ins).__name__ == "InstDMACopy"
            and ins.engine == mybir.EngineType.SP
        ):
            si = ins.sync_info
            if si is not None and si.on_wait:
                break
            moved.append(ins)
            if len(moved) >= HOIST_IN_DMAS:
                break
    # the Ln act-table load has no dependencies; pre-barrier it never gates
    # the first data-dependent activation
    for ins in list(body.instructions):
        if type(ins).__name__ == "InstLoadActFuncSet":
            si = ins.sync_info
            if si is None or not si.on_wait:
                moved.insert(0, ins)
            break
    for ins in moved:
        body.instructions.remove(ins)
    # position 1: after the leading dummy InstCall, before the barrier drains
    for k, ins in enumerate(moved):
        b0.instructions.insert(1 + k, ins)


def build(bins: np.ndarray):
    key = _constants(bins)
    if key is None:
        raise NotImplementedError("unsupported bins for this kernel")
    if key not in _BUILD_CACHE:
        _BUILD_CACHE[key] = _build(*key)
    return _BUILD_CACHE[key]


def make_in_maps(Xs: np.ndarray):
    shards = Xs.reshape(NCORES, P, W)
    return [{"xs": shards[c]} for c in range(NCORES)]


def kernel(Xs: np.ndarray, bins: np.ndarray) -> np.ndarray:
    Xs = np.asarray(Xs, dtype=np.float32)
    bins = np.asarray(bins, dtype=np.float32)
    nc = build(bins)
    res = run_bass_kernel_spmd(nc, make_in_maps(Xs), core_ids=list(range(NCORES)))
    out = np.concatenate([r["out"].reshape(-1) for r in res.results])
    return out.astype(np.float32)


# revision 45
# speedup vs baseline: 1.0554x; 1.0042x over previous
"""Trainium2 Bass kernel for nn_LogOddsPerformanceTransformer.

Computes, for each element x of Xs:
    s   = log(x) - log(1-x)              (log-odds)
    idx = clip(searchsorted(bins, max(s, bins[0]), 'right') - 1, 0, NB-1)
    out = bins[idx]

bins is a uniform grid (linspace), so binning reduces to an affine floor
via the magic-number rounding trick.  The post-log chain is one fused
6-stage custom-DVE op producing the integer bin offset j, plus one
2-ALU tensor_scalar (on gpsimd) for the final affine:

    OP1:  j   = ((clip((a-b)*inv, -31, 32) + M) - M)   # M = 2^23+31
    TS:   out = (j - 0.5) * step

Clamp bounds -31/32 (instead of the exact bin edges -31.5/32.5) keep
sg + 31 >= 0 so the magic add always lands on the integer rounding grid
at 2^23; any clamp value inside the first/last bin gives the identical
bin index.

Data parallel over 8 NeuronCores; per core the 524288-element slice is
viewed as [128 x 4096].  Input DMAs (SP/HWDGE) use a ramped column-tile
grid so the activation engine is never starved; compute runs on an
independent column-chunk grid (ACT 2x Ln -> DVE fused op -> Pool TS);
output DMAs issue per compute chunk on the SP sequencer, which is idle
after the input DMAs and whose in-order semaphore waits match the chunk
completion order.
"""

import sys

sys.path.insert(0, "/opt/trn_rl_repo")

from contextlib import ExitStack

import numpy as np

import concourse.bass as bass
import concourse.tile as tile
from concourse import bacc, mybir
from concourse.bass_utils import run_bass_kernel_spmd

N = 4_194_304
NCORES = 8
NPER = N // NCORES  # 524288
P = 128
W = NPER // P  # 4096 columns per core

# --- tunables -------------------------------------------------------------
IN_TILES = (128, 256, 512, 896, 1152, 1152)  # ramped; sum = 4096
# compute grid; sum = 4096.  Chunk ends should align under tile prefix sums
# so a chunk never waits on a tile it doesn't cover.
CHUNKS = (128, 256, 512, 896, 672, 608, 480, 288, 256)
# groups of chunk indices sharing one a-pass / b-pass (Ln) activation op:
# merging late-kernel ops (whose data has long arrived) saves the ~185ns
# per-instruction activation init without hurting the pipeline ramp.
A_GROUPS = tuple((i,) for i in range(len(CHUNKS)))
B_GROUPS = tuple((i,) for i in range(len(CHUNKS)))
# out-DMA grid; boundaries must be a subset of the chunk prefix sums.
OUT_TILES = CHUNKS
# per-out issue engine names; None -> all "sync".  The second-to-last out on
# "scalar" (ACT sequencer, idle by then) overlaps the last one's SP issue.
OUT_ENGINES = ("sync",) * 7 + ("scalar", "sync")
TAIL_TS_ON_DVE = 99  # last k chunks run the final tensor_scalar on DVE not Pool
TAIL_HIPRI = 0  # unused; kept for sweep-script compat
# Chunks before the last TAIL_F32 write their result in bf16: with j (exact
# small integers, bf16-lossless) also bf16, the final tensor_scalar runs in
# the DVE 2x perf mode (2 elem/cycle).  bf16 outputs ship via gpsimd
# cast-DMAs (bf16->f32, same modeled transfer time); the rounding of the
# final value adds ~1.6e-3 norm-rel error, well inside the 2e-2 gate.  The
# last TAIL_F32 chunks stay f32 so their outs use the low-latency HWDGE
# path.  Set TAIL_F32 >= len(CHUNKS) to disable bf16 entirely.  (Measured:
# the gpsimd cast-DMA issue latency outweighs the 2x TS win -> disabled.)
TAIL_F32 = 99
# hoist the first k input DMAs ahead of the framework start barrier (they
# have no dependencies); the barrier then overlaps the first HWDGE issues.
HOIST_IN_DMAS = 4
HOIST_TABLE = True  # post-compile: move the act-table load pre-barrier
WARMUP = False  # warmup activation unneeded once the table load is hoisted
STRIP_EXIT_BARRIER = True  # drop the redundant second exit barrier round
STRIP_START_BARRIER = True  # drop the start all-engine barrier (sem-ordered body)
# --------------------------------------------------------------------------

f32 = mybir.dt.float32
Alu = mybir.AluOpType
Act = mybir.ActivationFunctionType

_BUILD_CACHE: dict[tuple, object] = {}


# --- custom DVE op --------------------------------------------------------
def _j_ref(in0, in1, s0, s1, imm2):
    f = np.float32
    d = (in0.astype(f) - in1.astype(f)).astype(f)
    sg = (d * f(s0)).astype(f)
    mx = np.maximum(sg, f(imm2)).astype(f)
    lat = f(f(1.0) - f(imm2))
    mn = np.minimum(mx, lat).astype(f)
    t1 = (mn + f(s1)).astype(f)
    return (t1 - f(s1)).astype(f)


def _register_ops():
    import concourse.dve_ops as dve_ops
    from concourse.dve_spec import (
        Spec,
        Src0,
        Src1,
        C0,
        C1,
        C2,
        One,
        maxx,
        minn,
        lower,
        _has_src1,
    )
    from concourse.dve_uop import DveOpSpec

    def reg(name, spec):
        if name in dve_ops._SUB_OPCODE_FOR_NAME:
            return next(op for op in dve_ops.OPS if op.name == name)
        row = max(dve_ops._SUB_OPCODE_FOR_NAME.values()) + 1
        assert row < 0x20
        dve_ops._SUB_OPCODE_FOR_NAME[name] = row
        shas = {}
        for ver in ("v3", "v4"):
            uops = lower(spec, ver=ver)
            shas[ver] = DveOpSpec(
                name=name, opcode=row, uops=uops, rd1_en=_has_src1(spec)
            ).sha(ver)
        op = dve_ops.DveOp(name, spec, subdim=False, uops_sha=shas)
        dve_ops.OPS.append(op)
        dve_ops.CUSTOM_DVE_SPECS[name] = spec
        return op

    # j = ((clip((a-b)*C0, C2, 1-C2) + C1) - C1);  C2 = -31 so 1-C2 = 32
    sg = (Src0 - Src1) * C0
    mn = minn(maxx(sg, C2), One - C2)
    body = (mn + C1) - C1
    return reg("LOGODDS_J_ANT", Spec(body=body, reference=_j_ref))


_OP1 = _register_ops()


def _constants(bins: np.ndarray):
    """Host-side constants; returns None if the fused path can't be used
    (non-uniform bins or grid where the magic offsets aren't exact)."""
    b64 = bins.astype(np.float64)
    nb = len(bins)
    if nb != 64:
        return None
    step = np.float32((b64[-1] - b64[0]) / (nb - 1))
    inv = np.float32((nb - 1) / (b64[-1] - b64[0]))
    # sigma = s*inv ; bin edges at sigma = b0*inv + k.  Require b0*inv = -31.5
    # (true for the symmetric linspace(-6,6,64) grid) so the fixed clamp
    # bounds/magic below are exact.
    if not np.isclose(float(b64[0]) * float(inv), -31.5, atol=1e-6):
        return None
    uniform = np.allclose(
        np.diff(b64), (b64[-1] - b64[0]) / (nb - 1), rtol=0, atol=1e-5
    )
    if not uniform:
        return None
    return (float(inv), float(step))


MAGIC = float(np.float32(2.0**23 + 31.0))
SIG_LO = -31.0  # imm2 of OP1; upper clamp is 1-imm2 = 32
HALF = 0.5


def _build(inv, step):
    assert sum(IN_TILES) == W and sum(CHUNKS) == W and sum(OUT_TILES) == W
    ccum = np.cumsum(CHUNKS)
    assert set(np.cumsum(OUT_TILES)) <= set(ccum), "OUT_TILES must nest in CHUNKS"

    return _build_body(inv, step)


def _retag_const_memsets(nc):
    """Strip the framework preamble's const-AP memsets when nothing in the
    kernel references those const tensors (this kernel passes all activation
    biases and tensor_scalar operands as its own APs or immediates).  The
    memsets otherwise gate the kernel start barrier by ~0.3-0.5us.  If any
    instruction does reference a const AP, fall back to retagging the memsets
    from Pool (95ns Q7 launch each) to the cheaper DVE engine."""
    fn = nc.m.functions[0]
    referenced = any(
        "memref='const" in str(arg)
        for bb in fn.blocks
        for ins in bb.instructions
        for arg in (getattr(ins, "ins", None) or [])
    )
    for bb in fn.blocks:
        dead = [
            ins
            for ins in list(bb.instructions)
            if type(ins).__name__ == "InstMemset"
            and "memref='const" in str(ins.outs[0])
        ]
        for ins in dead:
            if referenced:
                if ins.engine == mybir.EngineType.Pool:
                    ins.engine = mybir.EngineType.DVE
            else:
                bb.instructions.remove(ins)


def _build_body(inv, step):
    nc = bacc.Bacc("TRN2", target_bir_lowering=False, debug=False)
    xs = nc.dram_tensor("xs", [P, W], f32, kind="ExternalInput").ap()
    outs = nc.dram_tensor("out", [P, W], f32, kind="ExternalOutput").ap()

    with tile.TileContext(nc) as tc, ExitStack() as ctx:
        tmp = ctx.enter_context(tc.tile_pool(name="tmp", bufs=1))

        bf16 = mybir.dt.bfloat16
        x = tmp.tile([P, W], f32, tag="x")
        a = tmp.tile([P, W], f32, tag="a")
        b = tmp.tile([P, W], f32, tag="b")
        j = tmp.tile([P, W], bf16, tag="j")
        o = tmp.tile([P, W], f32, tag="o")
        ob = tmp.tile([P, W], bf16, tag="ob")

        # all input DMAs issued first (high priority) so the out DMAs never
        # starve later input tiles
        with tc.high_priority():
            off = 0
            for w in IN_TILES:
                sl = (slice(None), slice(off, off + w))
                nc.sync.dma_start(x[sl], xs[sl])
                off += w

        # scalar constants built with DVE memsets (idle engine) so no
        # const-pool Memset gates the start barrier
        bias0 = tmp.tile([P, 1], f32, tag="bias0")
        bias1 = tmp.tile([P, 1], f32, tag="bias1")
        half_ap = tmp.tile([P, 1], f32, tag="half")
        step_ap = tmp.tile([P, 1], f32, tag="step")
        nc.vector.memset(bias0[:], 0.0)
        nc.vector.memset(bias1[:], 1.0)
        nc.vector.memset(half_ap[:], HALF)
        nc.vector.memset(step_ap[:], step)
        if WARMUP:
            # warmup: forces the Ln act-table load to run during the DMA
            # ramp instead of gating the first real activation
            warm = tmp.tile([P, 1], f32, tag="warm")
            nc.scalar.activation(warm[:], bias1[:], Act.Ln, bias0[:])

        NCH = len(CHUNKS)
        ccum = [0] + list(np.cumsum(CHUNKS))
        assert sorted(i for g in A_GROUPS for i in g) == list(range(NCH))
        assert sorted(i for g in B_GROUPS for i in g) == list(range(NCH))
        a_first = {g[0]: g for g in A_GROUPS}
        b_first = {g[0]: g for g in B_GROUPS}
        out_cum = list(np.cumsum(OUT_TILES))
        for ci in range(NCH):
            if ci in a_first:
                g = a_first[ci]
                gsl = (slice(None), slice(ccum[g[0]], ccum[g[-1] + 1]))
                nc.scalar.activation(a[gsl], x[gsl], Act.Ln, bias0[:])
            if ci in b_first:
                g = b_first[ci]
                gsl = (slice(None), slice(ccum[g[0]], ccum[g[-1] + 1]))
                nc.scalar.activation(b[gsl], x[gsl], Act.Ln, bias1[:], -1.0)
            off, off2 = ccum[ci], ccum[ci + 1]
            sl = (slice(None), slice(off, off2))
            is_bf16 = ci < NCH - TAIL_F32
            nc.vector._custom_dve(
                _OP1, out=j[sl], in0=a[sl], in1=b[sl], s0=inv, s1=MAGIC, imm2=SIG_LO
            )
            ts_eng = nc.vector if ci >= NCH - TAIL_TS_ON_DVE else nc.gpsimd
            ts_eng.tensor_scalar(
                (ob if is_bf16 else o)[sl],
                j[sl],
                half_ap[:],
                step_ap[:],
                Alu.subtract,
                Alu.mult,
            )
            if off2 in out_cum:
                oi = out_cum.index(off2)
                prev = 0 if oi == 0 else out_cum[oi - 1]
                osl = (slice(None), slice(prev, off2))
                if is_bf16:
                    nc.gpsimd.dma_start(outs[osl], ob[osl])
                else:
                    eng = "sync" if OUT_ENGINES is None else OUT_ENGINES[oi]
                    getattr(nc, eng).dma_start(outs[osl], o[osl])

    _retag_const_memsets(nc)
    _hoist_in_dmas(nc)
    nc.compile()
    if HOIST_TABLE:
        _hoist_act_table_load(nc)
    if STRIP_EXIT_BARRIER:
        _strip_second_exit_barrier(nc)
    if STRIP_START_BARRIER:
        _strip_start_barrier(nc)
    return nc


def _strip_start_barrier(nc):
    """Remove the start all-engine barrier (Drain + EventSemaphore pairs in
    block 0).  After the const-memset strip the preamble contains nothing the
    body depends on through anything but explicit semaphores: cross-engine
    body ordering is fully semaphore-based (tile framework), and same-engine
    program order covers the hoisted DMAs / act-table load.  Each engine can
    launch straight into its stream."""
    b0 = list(nc.m.functions[0].blocks)[0]
    dead = [
        ins
        for ins in list(b0.instructions)
        if type(ins).__name__ in ("InstDrain", "InstEventSemaphore")
    ]
    for ins in dead:
        b0.instructions.remove(ins)


def _strip_second_exit_barrier(nc):
    """The compiled epilogue runs two all-engine barrier rounds separated by
    the Pool dma_reset ISA op.  The second round only re-synchronizes the
    engines after that reset; every engine's stream may simply end at round
    one (the runtime waits for each queue to drain independently), so drop
    round two (~0.3us off the kernel end)."""
    blocks = list(nc.m.functions[0].blocks)
    epi = blocks[-1]
    ins_list = list(epi.instructions)
    # find the Pool ISA (dma_reset) marker; everything after it that is a
    # Drain/EventSemaphore barrier pair is round two
    isa_idx = None
    for i, ins in enumerate(ins_list):
        if type(ins).__name__ == "InstISA" and ins.engine == mybir.EngineType.Pool:
            isa_idx = i
    if isa_idx is None:
        return
    tail = ins_list[isa_idx + 1 :]
    if tail and all(
        type(t).__name__ in ("InstDrain", "InstEventSemaphore") for t in tail
    ):
        for t in tail:
            epi.instructions.remove(t)


def _hoist_act_table_load(nc):
    """Post-compile: move the (dependency-free) Ln act-table load ahead of
    the start-barrier drains so its 1.3us never gates the first
    data-dependent activation.  It must run post-compile because the BIR
    lowering inserts InstLoadActFuncSet lazily before the first
    activation."""
    blocks = list(nc.m.functions[0].blocks)
    if len(blocks) < 2:
        return
    b0, body = blocks[0], blocks[1]
    for ins in list(body.instructions):
        if type(ins).__name__ == "InstLoadActFuncSet":
            si = ins.sync_info
            if si is None or not si.on_wait:
                body.instructions.remove(ins)
                b0.instructions.insert(1, ins)
            return


def _hoist_in_dmas(nc):
    """Move the first HOIST_IN_DMAS input DMACopy instructions (SP engine,
    no semaphore waits) to the front of the first block, ahead of the
    framework's start-barrier drains.  Their HWDGE issue then overlaps the
    barrier instead of waiting for it, pulling the whole pipeline earlier.
    Per-engine program order is preserved (they were SP's first body
    instructions)."""
    if HOIST_IN_DMAS <= 0:
        return
    blocks = list(nc.m.functions[0].blocks)
    if len(blocks) < 2:
        return
    b0, body = blocks[0], blocks[1]
    moved = []
    for ins in list(body.instructions):
        if (
            type(ins).__name__ == "InstDMACopy"
            and ins.engine == mybir.EngineType.SP
        ):
            si = ins.sync_info
            if si is not None and si.on_wait:
                break
            moved.append(ins)
            if len(moved) >= HOIST_IN_DMAS:
                break
    # the Ln act-table load has no dependencies; pre-barrier it never gates
    # the first data-dependent activation
    for ins in list(body.instructions):
        if type(ins).__name__ == "InstLoadActFuncSet":
            si = ins.sync_info
            if si is None or not si.on_wait:
                moved.insert(0, ins)
            break
    for ins in moved:
        body.instructions.remove(ins)
    # position 1: after the leading dummy InstCall, before the barrier drains
    for k, ins in enumerate(moved):
        b0.instructions.insert(1 + k, ins)


def build(bins: np.ndarray):
    key = _constants(bins)
    if key is None:
        raise NotImplementedError("unsupported bins for this kernel")
    if key not in _BUILD_CACHE:
        _BUILD_CACHE[key] = _build(*key)
    return _BUILD_CACHE[key]


def make_in_maps(Xs: np.ndarray):
    shards = Xs.reshape(NCORES, P, W)
    return [{"xs": shards[c]} for c in range(NCORES)]


def kernel(Xs: np.ndarray, bins: np.ndarray) -> np.ndarray:
    Xs = np.asarray(Xs, dtype=np.float32)
    bins = np.asarray(bins, dtype=np.float32)
    nc = build(bins)
    res = run_bass_kernel_spmd(nc, make_in_maps(Xs), core_ids=list(range(NCORES)))
    out = np.concatenate([r["out"].reshape(-1) for r in res.results])
    return out.astype(np.float32)
